# revision 4
# baseline (speedup 1.0000x reference)
"""Trainium2 Bass kernel for DigitsCapsule dynamic routing.

Strategy (8 NeuronCores, data-parallel over batch B=512 -> 64 per core):
  u_hat = einsum('BIk,IklO->BIlO', x, w) is NEVER materialized (264 MB).
  Instead, per routing iteration:
    s    = x @ (e ⊙ w)              (PE matmul over r=(k,I)=9216, e = unnorm. softmax weights)
    v    = squash(s / S[l])          (softmax normalizer folded into squash)
    T2   = xᵀ @ v                    (PE outer-product accumulation)
    u_vj = Σ_{k,O} w ⊙ T2            (copy/mult/reduce spread over Act+DVE+Pool)
    b   += AllReduce(u_vj)           (mean over full B via 8-core collective)
  Last iteration skips the u_vj/collective (dead in the reference).

Engine balance: the agreement step's PSUM->SBUF copies run on Activation,
the w⊙T2 multiplies on DVE (2x fp16), and the (k,O) reductions mostly on
the otherwise-idle Pool engine (TensorReduce has no DVE fast mode, so Pool
at 0.6 efficiency is nearly competitive). sqrt(x) is computed as
exp(0.5·ln x) so every activation function (Exp/Ln/Copy/Square) lives in
the one 'natural_log_exp_and_others' table set — no LoadActFuncSet thrash.

Row space r = k*1152 + I (k-major); free space f = l*7 + O (w's natural order).
All layout permutes are done host-side in numpy; the device program has zero
transposes.
"""

import numpy as np

B, I, K, L, O = 512, 1152, 8, 16, 7
NC = 8
BL = B // NC          # 64 batch rows per core
R = K * I             # 9216
F = L * O             # 112
NI = I // 128         # 9 partition chunks of I
ITERS = 3

_CACHE = {}

# per-i engine assignment for the agreement step:
#   copy: 'act' | 'dve' | 'pool' | None (None = direct mult from PSUM, no copy)
#   mult: 'dve' | 'pool'   (ignored when copy is None on the same engine rules)
#   red:  'dve' | 'pool'
DEF_CFG = {
    "copy": ("act",) * 9,
    "mult": ("dve",) * 9,
    "red": ("dve",) * 9,
    "warm": 14,
}


def _build(dt_key, repeat=1, abl=(), cfg=None):
    """abl: ablation flags for benchmarking — subsets of
    {"no_ar", "no_u", "no_wc", "no_smm", "no_warm"}."""
    import concourse.bacc as bacc
    import concourse.mybir as mybir
    import concourse.tile as tile

    cfg = dict(DEF_CFG, **(cfg or {}))

    DT = {"f32": mybir.dt.float32, "f16": mybir.dt.float16}[dt_key]
    F32 = mybir.dt.float32
    AF = mybir.ActivationFunctionType
    ALU = mybir.AluOpType
    AX = mybir.AxisListType

    nc = bacc.Bacc("TRN2", target_bir_lowering=False, debug=False, num_devices=NC)

    x_nat_d = nc.dram_tensor("x_nat", [BL, R], DT, kind="ExternalInput")
    x_T_d = nc.dram_tensor("x_T", [R, BL], DT, kind="ExternalInput")
    w2_d = nc.dram_tensor("w2", [I, K * F], DT, kind="ExternalInput")
    y_d = nc.dram_tensor("y", [BL, O, L], F32, kind="ExternalOutput")

    with tile.TileContext(nc) as tc:
        with (
            tc.tile_pool(name="const", bufs=1) as cpool,
            tc.tile_pool(name="work", bufs=2) as wpool,
            tc.tile_pool(name="wc", bufs=10) as wcfull,
            tc.tile_pool(name="step6", bufs=4) as wcpool,
            tc.tile_pool(name="small", bufs=2) as spool,
            tc.tile_pool(name="ps_s", bufs=1, space="PSUM") as ps_s,
            tc.tile_pool(name="ps_t2", bufs=3, space="PSUM") as ps_t2,
            tc.tile_pool(name="ps_sm", bufs=1, space="PSUM") as ps_sm,
            tc.tile_pool(name="dram", bufs=2, space="DRAM") as dpool,
        ):
            ENG = {"dve": nc.vector, "pool": nc.gpsimd}

            # ---- load inputs (issue spread across engine DGE queues) ----
            x_nat = cpool.tile([BL, R], DT, tag="x_nat")
            for h in range(2):
                nc.gpsimd.dma_start(x_nat[:, h * R // 2:(h + 1) * R // 2],
                                    x_nat_d[:, h * R // 2:(h + 1) * R // 2])

            # xT tiles: slot t=(k*9+i) holds rows k*1152+i*128 .. +128
            NT = K * NI
            xT = cpool.tile([128, NT * BL], DT, tag="xT")
            xt_src = x_T_d[:].rearrange("(t p) b -> p t b", p=128)
            xt_dst = xT[:].rearrange("p (t b) -> p t b", t=NT)
            for h in range(4):
                lo, hi = h * NT // 4, (h + 1) * NT // 4
                nc.sync.dma_start(xt_dst[:, lo:hi], xt_src[:, lo:hi])

            w2 = cpool.tile([128, NI * K * F], DT, tag="w2")
            w2_src = w2_d[:].rearrange("(i p) f -> p i f", p=128)
            w2_dst = w2[:].rearrange("p (i f) -> p i f", i=NI)
            for h in range(3):
                lo, hi = h * 3, (h + 1) * 3
                nc.scalar.dma_start(w2_dst[:, lo:hi], w2_src[:, lo:hi])

            ones = cpool.tile([128, 1], DT, tag="ones")
            nc.vector.memset(ones[:], 1.0)
            ones64 = cpool.tile([1, BL], F32, tag="ones64")
            nc.vector.memset(ones64[:], 1.0)

            # warm the PE clock-gate during the input-DMA phase so iteration
            # 0's matmuls run at 2.4 GHz (same proven shape as the AR-window
            # keep-warm below; results never read)
            if "no_warm" not in abl:
                warm0 = cpool.tile([128, NI * L], DT, tag="warm0")
                nc.vector.memset(warm0[:], 0.0)
                wt0 = ps_sm.tile([BL, NI * L + L], F32, tag="sm")
                for _ in range(cfg["warm"]):
                    nc.tensor.matmul(wt0[0:1, 0:NI * L], ones[:], warm0[:],
                                     start=True, stop=True)


            b_acc = cpool.tile([128, NI * L], F32, tag="b_acc")

            def w2_i(i):
                return w2[:, i * K * F:(i + 1) * K * F]

            for rep in range(repeat):
             for t in range(ITERS):
                # ---- coupling coefficients (unnormalized) ----
                if t == 0:
                    e9 = None         # e == 1 -> Wc == w2, invS == 1/1152
                    invS_b = None
                else:
                    # e replicated over O (contiguous innermost for Wc mult),
                    # in 2 chunks so Wc_0 starts before the full exp is done
                    e9 = wpool.tile([128, NI * F], DT, tag="e9")
                    for lo, hi in ((0, 5), (5, NI)):
                        nc.scalar.activation(
                            e9[:, lo * F:hi * F]
                            .rearrange("p (jl o) -> p jl o", o=O),
                            b_acc[:, lo * L:hi * L]
                            .unsqueeze(2).to_broadcast((128, (hi - lo) * L, O)),
                            AF.Exp, scale=1.0 / B)
                    # compact e for the column-sum S (N<=512 matmul limit)
                    e_nat = wpool.tile([128, NI * L], DT, tag="e_nat")
                    nc.scalar.activation(e_nat[:], b_acc[:], AF.Exp, scale=1.0 / B)
                    sm_ps = ps_sm.tile([BL, NI * L + L], F32, tag="sm")
                    ssum = sm_ps[0:1, 0:NI * L]
                    nc.tensor.matmul(ssum, ones[:], e_nat[:], start=True, stop=True)
                    S16 = spool.tile([1, L], F32, tag="S16")
                    nc.vector.tensor_reduce(
                        S16[:], ssum.rearrange("p (i l) -> p l i", i=NI),
                        axis=AX.X, op=ALU.add)
                    invS16 = spool.tile([1, L], F32, tag="invS16")
                    nc.vector.reciprocal(invS16[:], S16[:])
                    bc_ps = sm_ps[0:BL, NI * L:NI * L + L]
                    nc.tensor.matmul(bc_ps, ones64[:], invS16[:],
                                     start=True, stop=True)
                    invS_b = spool.tile([BL, L], F32, tag="invS_b")
                    nc.vector.tensor_copy(invS_b[:], bc_ps)

                # ---- s = x @ (e*w) ----
                s_ps = ps_s.tile([BL, F], F32, tag="s_ps")
                if "no_smm" in abl:
                    zz = spool.tile([BL, F], F32, tag="zz")
                    nc.vector.memset(zz[:], 0.0)
                    nc.scalar.activation(s_ps[:], zz[:], AF.Copy)
                wcs = []
                for i in range(NI):
                    if "no_smm" in abl:
                        break
                    if t == 0 or "no_wc" in abl:
                        wcs.append(w2_i(i))
                    else:
                        wc = wcfull.tile([128, K * F], DT, tag="wc")
                        e_b = (e9[:, i * F:(i + 1) * F]
                               .unsqueeze(1).to_broadcast((128, K, F)))
                        nc.vector.tensor_tensor(
                            wc[:].rearrange("p (k f) -> p k f", k=K),
                            w2_i(i).rearrange("p (k f) -> p k f", k=K),
                            e_b, op=ALU.mult)
                        wcs.append(wc[:])
                # k-outer matches the xT DMA arrival order (t = k*9+i)
                if "no_smm" not in abl:
                    for k in range(K):
                        for i in range(NI):
                            tslot = k * NI + i
                            nc.tensor.matmul(
                                s_ps[:],
                                xT[:, tslot * BL:(tslot + 1) * BL],
                                wcs[i][:, k * F:(k + 1) * F],
                                start=(i == 0 and k == 0),
                                stop=(i == NI - 1 and k == K - 1))

                # ---- squash (with 1/S[l] folded in) ----
                s_n = wpool.tile([BL, F], F32, tag="s_n")
                if t == 0:
                    nc.vector.tensor_scalar_mul(s_n[:], s_ps[:], 1.0 / I)
                else:
                    nc.vector.tensor_tensor(
                        s_n[:].rearrange("p (l o) -> p l o", o=O),
                        s_ps[:].rearrange("p (l o) -> p l o", o=O),
                        invS_b[:].unsqueeze(2).to_broadcast((BL, L, O)),
                        op=ALU.mult)
                # squash factor: sq/((1+sq)*sqrt(sq)) == sqrt(sq)/(1+sq);
                # sqrt via exp(0.5*ln) keeps Act in the Exp/Ln/Copy table set
                sq2 = wpool.tile([BL, F], F32, tag="sq2")
                nc.scalar.square(sq2[:], s_n[:])
                sq = spool.tile([BL, L], F32, tag="sq")
                nc.vector.tensor_reduce(
                    sq[:], sq2[:].rearrange("p (l o) -> p l o", o=O),
                    axis=AX.X, op=ALU.add)
                lnq = spool.tile([BL, L], F32, tag="lnq")
                nc.scalar.activation(lnq[:], sq[:], AF.Ln)
                nrm = spool.tile([BL, L], F32, tag="nrm")
                nc.scalar.activation(nrm[:], lnq[:], AF.Exp, scale=0.5)
                d1 = spool.tile([BL, L], F32, tag="d1")
                nc.scalar.add(d1[:], sq[:], 1.0)
                rin = spool.tile([BL, L], F32, tag="rin")
                nc.vector.reciprocal(rin[:], d1[:])
                fm = spool.tile([BL, L], F32, tag="fm")
                nc.vector.tensor_tensor(fm[:], nrm[:], rin[:], op=ALU.mult)
                if t == ITERS - 1:
                    # fuse the (l,o)->(o,l) output permute into the v multiply
                    v_out = wpool.tile([BL, F], F32, tag="v_out")
                    nc.vector.tensor_tensor(
                        v_out[:].rearrange("p (o l) -> p l o", o=O),
                        s_n[:].rearrange("p (l o) -> p l o", o=O),
                        fm[:].unsqueeze(2).to_broadcast((BL, L, O)),
                        op=ALU.mult)
                    nc.sync.dma_start(y_d[:], v_out[:])
                    continue

                # v in fp16 feeds the T2 matmuls directly
                v_sb = wpool.tile([BL, F], DT, tag="v_sb")
                nc.vector.tensor_tensor(
                    v_sb[:].rearrange("p (l o) -> p l o", o=O),
                    s_n[:].rearrange("p (l o) -> p l o", o=O),
                    fm[:].unsqueeze(2).to_broadcast((BL, L, O)),
                    op=ALU.mult)

                # ---- agreement: u = sum_{k,O} w * (x^T v) ----
                if "no_u" in abl:
                    if t == 0:
                        nc.vector.memset(b_acc[:], 0.0)
                    continue
                v16 = v_sb
                # agreement tensor in DT (fp16 halves AR payload + bounce DMAs)
                u_nat = wpool.tile([128, NI * L], DT, tag="u_nat")
                for i in range(NI):
                    t2c = ps_t2.tile([128, 1024], F32, tag="t2")
                    for k in range(K):
                        nc.tensor.matmul(
                            t2c[:, k * 128:k * 128 + F],
                            x_nat[:, k * I + i * 128:k * I + (i + 1) * 128],
                            v16[:], start=True, stop=True)
                    t2v = (t2c[:].rearrange("p (k x) -> p k x", k=K)
                           [:, :, 0:F])
                    prod = wcpool.tile([128, K * F], DT, tag="prod")
                    ceng = cfg["copy"][i]
                    if ceng is None:
                        ENG[cfg["mult"][i]].tensor_tensor(
                            prod[:].rearrange("p (k f) -> p k f", k=K),
                            t2v,
                            w2_i(i).rearrange("p (k f) -> p k f", k=K),
                            op=ALU.mult)
                    else:
                        t2s = wcpool.tile([128, K * F], DT, tag="t2s")
                        t2s_v = t2s[:].rearrange("p (k f) -> p k f", k=K)
                        if ceng == "act":
                            nc.scalar.activation(t2s_v, t2v, AF.Copy)
                        else:
                            ENG[ceng].tensor_copy(t2s_v, t2v)
                        ENG[cfg["mult"][i]].tensor_tensor(
                            prod[:], t2s[:], w2_i(i), op=ALU.mult)
                    with nc.allow_low_precision("fp16 agreement; b re-acc fp32"):
                        ENG[cfg["red"][i]].tensor_reduce(
                            u_nat[:, i * L:(i + 1) * L],
                            prod[:].rearrange("p (k l o) -> p l k o", k=K, l=L),
                            axis=AX.XY, op=ALU.add)

                ar_in = dpool.tile([128, NI * L], DT, tag="ar_in")
                ar_out = dpool.tile([128, NI * L], DT, tag="ar_out")
                # bounce in thirds so the DMA overlaps the tail reduces
                for h in range(3):
                    lo, hi = h * 3 * L, (h + 1) * 3 * L
                    nc.sync.dma_start(ar_in[:, lo:hi], u_nat[:, lo:hi])
                if "no_ar" in abl:
                    nc.sync.dma_start(ar_out[:], ar_in[:])
                else:
                    nc.gpsimd.collective_compute(
                        "AllReduce", ALU.add,
                        replica_groups=[list(range(NC))],
                        ins=[ar_in.opt()], outs=[ar_out.opt()])
                # keep PE's HAM clock-gate open through the ~10us collective:
                # dummy column-sums of u_nat (same proven shape as the ssum
                # matmul), never read; reading u_nat pins them to this window
                if "no_warm" not in abl:
                    wt = ps_sm.tile([BL, NI * L + L], F32, tag="sm")
                    for _ in range(cfg["warm"]):
                        nc.tensor.matmul(wt[0:1, 0:NI * L], ones[:], u_nat[:],
                                         start=True, stop=True)
                u_allr = wpool.tile([128, NI * L], DT, tag="u_allr")
                nc.sync.dma_start(u_allr[:], ar_out[:])
                if t == 0:
                    nc.vector.tensor_copy(b_acc[:], u_allr[:])
                else:
                    nc.vector.tensor_add(b_acc[:], b_acc[:], u_allr[:])

    nc.compile()
    return nc


def _freeze(cfg):
    if not cfg:
        return ()
    return tuple(sorted((k, tuple(v) if isinstance(v, (list, tuple)) else v)
                        for k, v in cfg.items()))


def _get_nc(dt_key, repeat=1, abl=(), cfg=None):
    key = (dt_key, repeat, tuple(sorted(abl)), _freeze(cfg))
    if key not in _CACHE:
        _CACHE[key] = _build(dt_key, repeat, abl, cfg)
    return _CACHE[key]


def kernel(x, w, _dt="f16", _trace=False):
    x = np.asarray(x, dtype=np.float32)
    w = np.asarray(w, dtype=np.float32)
    np_dt = {"f32": np.float32, "f16": np.float16}[_dt]

    nc = _get_nc(_dt)

    w2 = np.ascontiguousarray(w.reshape(I, K * F).astype(np_dt))
    in_maps = []
    for c in range(NC):
        xv = x[c * BL:(c + 1) * BL]
        x_nat = np.ascontiguousarray(
            xv.transpose(0, 2, 1).astype(np_dt)).reshape(BL, R)
        x_T = np.ascontiguousarray(
            xv.transpose(2, 1, 0).astype(np_dt)).reshape(R, BL)
        in_maps.append({"x_nat": x_nat, "x_T": x_T, "w2": w2})

    from concourse.bass_utils import run_bass_kernel_spmd
    res = run_bass_kernel_spmd(
        nc, in_maps, core_ids=list(range(NC)), trace=_trace)
    kernel.last_result = res
    out = np.concatenate([res.results[c]["y"] for c in range(NC)], axis=0)
    return out.astype(np.float32)


kernel.last_result = None


# revision 7
# speedup vs baseline: 1.1422x; 1.1422x over previous
"""Trainium2 Bass kernel for DigitsCapsule dynamic routing — transposed
agreement step + split, compute-overlapped AllReduce.

Per routing iteration:
    s    = x @ (e ⊙ w)               (PE, i-outer so Wc production overlaps)
    v    = squash(s / S[l])           (softmax normalizer folded into squash)
    T2ᵀ  = vᵀ-stationary outer-prod   (PE; out = [(l,o)=112 partitions, (k,i)],
                                       2 strided 512-col matmuls per i-chunk)
    prodᵀ= w2ᵀ ⊙ T2ᵀ                  (per half-chunk: Act/Pool copy + DVE 2x
                                       mult, or direct 1x mult from PSUM)
    u    = Σ_{(l,o)} prodᵀ · Sel      (accumulating PE matmuls against a 0/1
                                       selector; output lands as [i-part, l])
    b   += AllReduce(mean_B u)        (one 8-core collective per iteration;
                                       fp16 [I,L] payload via DRAM bounce)
"""

import numpy as np

B, I, K, L, O = 512, 1152, 8, 16, 7
NC = 8
BL = B // NC          # 64 batch rows per core
R = K * I             # 9216
F = L * O             # 112
NI = I // 128         # 9 partition chunks of I
NH = 2 * NI           # 18 half-chunks (4 k's each)
ITERS = 3

_CACHE = {}

# path per half-chunk (chunk c, half h -> index 2c+h). Pool/DMA cannot
# touch PSUM, so only Act/DVE can evacuate the T2 matmul output:
#   'a' = Act copy + DVE mult       'b' = Act copy + Pool mult
#   'd' = DVE direct mult (PSUM)
DEF_CFG = {
    "path": "abdadbabdadbabdbdd",
    "warm": 14,
    "warmw": 144,
    # one full-width collective per iteration: the NRT AllReduce has a
    # ~5.5us fixed cost, so two half-collectives measured strictly slower
    "split_ar": False,
    "wc_pool": 0,      # how many of the 9 Wc chunks run on Pool
}


def _build(dt_key, repeat=1, abl=(), cfg=None):
    """abl: ablation flags for benchmarking — subsets of
    {"no_ar", "no_u", "no_wc", "no_smm", "no_warm"}."""
    import concourse.bacc as bacc
    import concourse.mybir as mybir
    import concourse.tile as tile

    cfg = dict(DEF_CFG, **(cfg or {}))
    assert len(cfg["path"]) == NH

    DT = {"f32": mybir.dt.float32, "f16": mybir.dt.float16}[dt_key]
    F32 = mybir.dt.float32
    AF = mybir.ActivationFunctionType
    ALU = mybir.AluOpType
    AX = mybir.AxisListType

    HALVES = ((0, 5), (5, NI)) if cfg["split_ar"] else ((0, NI),)
    FLUSH = {hi: (idx, lo, hi) for idx, (lo, hi) in enumerate(HALVES)}

    nc = bacc.Bacc("TRN2", target_bir_lowering=False, debug=False, num_devices=NC)

    x_nat_d = nc.dram_tensor("x_nat", [BL, R], DT, kind="ExternalInput")
    x_T_d = nc.dram_tensor("x_T", [R, BL], DT, kind="ExternalInput")
    w2_d = nc.dram_tensor("w2", [I, K * F], DT, kind="ExternalInput")
    w2t_d = nc.dram_tensor("w2t", [F, NI * K * 128], DT, kind="ExternalInput")
    sel_d = nc.dram_tensor("sel", [F, L], DT, kind="ExternalInput")
    y_d = nc.dram_tensor("y", [BL, O, L], F32, kind="ExternalOutput")

    with tile.TileContext(nc) as tc:
        with (
            tc.tile_pool(name="const", bufs=1) as cpool,
            tc.tile_pool(name="work", bufs=2) as wpool,
            tc.tile_pool(name="wc", bufs=10) as wcfull,
            tc.tile_pool(name="step6", bufs=4) as zpool,
            tc.tile_pool(name="small", bufs=2) as spool,
            tc.tile_pool(name="ps_s", bufs=1, space="PSUM") as ps_s,
            tc.tile_pool(name="ps_t2", bufs=4, space="PSUM") as ps_t2,
            tc.tile_pool(name="ps_u", bufs=1, space="PSUM") as ps_u,
            tc.tile_pool(name="ps_sm", bufs=1, space="PSUM") as ps_sm,
            tc.tile_pool(name="dram", bufs=2, space="DRAM") as dpool,
        ):
            # ---- load inputs (issue spread across engine DGE queues) ----
            x_nat = cpool.tile([BL, R], DT, tag="x_nat")
            for h in range(2):
                nc.gpsimd.dma_start(x_nat[:, h * R // 2:(h + 1) * R // 2],
                                    x_nat_d[:, h * R // 2:(h + 1) * R // 2])

            # xT tiles: slot t=(k*9+i) holds rows k*1152+i*128 .. +128
            NT = K * NI
            xT = cpool.tile([128, NT * BL], DT, tag="xT")
            xt_src = x_T_d[:].rearrange("(t p) b -> p t b", p=128)
            xt_dst = xT[:].rearrange("p (t b) -> p t b", t=NT)
            for h in range(4):
                lo, hi = h * NT // 4, (h + 1) * NT // 4
                nc.sync.dma_start(xt_dst[:, lo:hi], xt_src[:, lo:hi])

            w2 = cpool.tile([128, NI * K * F], DT, tag="w2")
            w2_src = w2_d[:].rearrange("(i p) f -> p i f", p=128)
            w2_dst = w2[:].rearrange("p (i f) -> p i f", i=NI)
            for h in range(3):
                lo, hi = h * 3, (h + 1) * 3
                nc.scalar.dma_start(w2_dst[:, lo:hi], w2_src[:, lo:hi])

            w2t = cpool.tile([F, NI * K * 128], DT, tag="w2t")
            for h in range(3):
                lo, hi = h * NI * K * 128 // 3, (h + 1) * NI * K * 128 // 3
                nc.scalar.dma_start(w2t[:, lo:hi], w2t_d[:, lo:hi])
            sel = cpool.tile([F, L], DT, tag="sel")
            nc.sync.dma_start(sel[:], sel_d[:])

            ones = cpool.tile([128, 1], DT, tag="ones")
            nc.vector.memset(ones[:], 1.0)
            ones64 = cpool.tile([1, BL], F32, tag="ones64")
            nc.vector.memset(ones64[:], 1.0)

            # warm the PE clock-gate during the input-DMA phase so iteration
            # 0's matmuls run at 2.4 GHz
            WW = cfg["warmw"]
            if "no_warm" not in abl:
                warm0 = cpool.tile([128, WW], DT, tag="warm0")
                nc.vector.memset(warm0[:], 0.0)
                wt0 = ps_sm.tile([BL, 512], F32, tag="sm")
                for _ in range(cfg["warm"]):
                    nc.tensor.matmul(wt0[0:1, 0:WW], ones[:], warm0[:],
                                     start=True, stop=True)


            b_acc = cpool.tile([128, NI * L], F32, tag="b_acc")

            def w2_i(i):
                return w2[:, i * K * F:(i + 1) * K * F]

            for rep in range(repeat):
             prev_out = None
             for t in range(ITERS):
                # ---- absorb previous AR (per half) + e9 + Wc ----
                if t == 0:
                    e9 = None         # e == 1 -> Wc == w2, invS == 1/1152
                    invS_b = None
                    wcs = [w2_i(i) for i in range(NI)]
                else:
                    u_allr = wpool.tile([128, NI * L], DT, tag="u_allr")
                    e9 = wpool.tile([128, NI * F], DT, tag="e9")
                    wcs = []
                    for idx, (lo, hi) in enumerate(HALVES):
                        cl, ch = lo * L, hi * L
                        nc.sync.dma_start(u_allr[:, cl:ch], prev_out[idx][:])
                        if t == 1:
                            nc.vector.tensor_copy(b_acc[:, cl:ch],
                                                  u_allr[:, cl:ch])
                        else:
                            nc.vector.tensor_add(b_acc[:, cl:ch],
                                                 b_acc[:, cl:ch],
                                                 u_allr[:, cl:ch])
                        # e replicated over O (contiguous innermost for Wc)
                        nc.scalar.activation(
                            e9[:, lo * F:hi * F]
                            .rearrange("p (jl o) -> p jl o", o=O),
                            b_acc[:, cl:ch]
                            .unsqueeze(2).to_broadcast((128, (hi - lo) * L, O)),
                            AF.Exp, scale=1.0 / B)
                        for i in range(lo, hi):
                            if "no_wc" in abl:
                                wcs.append(w2_i(i))
                                continue
                            wc = wcfull.tile([128, K * F], DT, tag="wc")
                            e_b = (e9[:, i * F:(i + 1) * F]
                                   .unsqueeze(1).to_broadcast((128, K, F)))
                            weng = (nc.gpsimd if i % NI < cfg["wc_pool"]
                                    else nc.vector)
                            weng.tensor_tensor(
                                wc[:].rearrange("p (k f) -> p k f", k=K),
                                w2_i(i).rearrange("p (k f) -> p k f", k=K),
                                e_b, op=ALU.mult)
                            wcs.append(wc[:])

                # ---- s = x @ (e*w), i-outer so it chases Wc production ----
                s_ps = ps_s.tile([BL, F], F32, tag="s_ps")
                if "no_smm" in abl:
                    zz = spool.tile([BL, F], F32, tag="zz")
                    nc.vector.memset(zz[:], 0.0)
                    nc.scalar.activation(s_ps[:], zz[:], AF.Copy)
                else:
                    for i in range(NI):
                        for k in range(K):
                            tslot = k * NI + i
                            nc.tensor.matmul(
                                s_ps[:],
                                xT[:, tslot * BL:(tslot + 1) * BL],
                                wcs[i][:, k * F:(k + 1) * F],
                                start=(i == 0 and k == 0),
                                stop=(i == NI - 1 and k == K - 1))

                # ---- softmax normalizer S (emitted after the s matmuls so
                # its PE ops don't block them in the in-order PE queue) ----
                if t > 0:
                    e_nat = wpool.tile([128, NI * L], DT, tag="e_nat")
                    nc.scalar.activation(e_nat[:], b_acc[:], AF.Exp,
                                         scale=1.0 / B)
                    sm_ps = ps_sm.tile([BL, 512], F32, tag="sm")
                    ssum = sm_ps[0:1, 0:NI * L]
                    nc.tensor.matmul(ssum, ones[:], e_nat[:],
                                     start=True, stop=True)
                    S16 = spool.tile([1, L], F32, tag="S16")
                    nc.vector.tensor_reduce(
                        S16[:], ssum.rearrange("p (i l) -> p l i", i=NI),
                        axis=AX.X, op=ALU.add)
                    invS16 = spool.tile([1, L], F32, tag="invS16")
                    nc.vector.reciprocal(invS16[:], S16[:])
                    bc_ps = sm_ps[0:BL, NI * L:NI * L + L]
                    nc.tensor.matmul(bc_ps, ones64[:], invS16[:],
                                     start=True, stop=True)
                    invS_b = spool.tile([BL, L], F32, tag="invS_b")
                    nc.scalar.copy(invS_b[:], bc_ps)

                # ---- squash (with 1/S[l] folded in) ----
                s_n = wpool.tile([BL, F], F32, tag="s_n")
                if t == 0:
                    nc.vector.tensor_scalar_mul(s_n[:], s_ps[:], 1.0 / I)
                else:
                    nc.vector.tensor_tensor(
                        s_n[:].rearrange("p (l o) -> p l o", o=O),
                        s_ps[:].rearrange("p (l o) -> p l o", o=O),
                        invS_b[:].unsqueeze(2).to_broadcast((BL, L, O)),
                        op=ALU.mult)
                # squash factor: sq/((1+sq)*sqrt(sq)) == sqrt(sq)/(1+sq)
                # (Act order per iter: Exp* -> Square -> [load] Sqrt -> Copy*)
                sq2 = wpool.tile([BL, F], F32, tag="sq2")
                nc.scalar.square(sq2[:], s_n[:])
                sq = spool.tile([BL, L], F32, tag="sq")
                nc.vector.tensor_reduce(
                    sq[:], sq2[:].rearrange("p (l o) -> p l o", o=O),
                    axis=AX.X, op=ALU.add)
                nrm = spool.tile([BL, L], F32, tag="nrm")
                nc.scalar.activation(nrm[:], sq[:], AF.Sqrt)
                d1 = spool.tile([BL, L], F32, tag="d1")
                nc.vector.tensor_scalar_add(d1[:], sq[:], 1.0)
                rin = spool.tile([BL, L], F32, tag="rin")
                nc.vector.reciprocal(rin[:], d1[:])
                fm = spool.tile([BL, L], F32, tag="fm")
                nc.vector.tensor_tensor(fm[:], nrm[:], rin[:], op=ALU.mult)
                if t == ITERS - 1:
                    # fuse the (l,o)->(o,l) output permute into the v multiply
                    v_out = wpool.tile([BL, F], F32, tag="v_out")
                    nc.vector.tensor_tensor(
                        v_out[:].rearrange("p (o l) -> p l o", o=O),
                        s_n[:].rearrange("p (l o) -> p l o", o=O),
                        fm[:].unsqueeze(2).to_broadcast((BL, L, O)),
                        op=ALU.mult)
                    nc.sync.dma_start(y_d[:], v_out[:])
                    continue

                # v in fp16: stationary operand for the T2^T matmuls
                v_sb = wpool.tile([BL, F], DT, tag="v_sb")
                nc.vector.tensor_tensor(
                    v_sb[:].rearrange("p (l o) -> p l o", o=O),
                    s_n[:].rearrange("p (l o) -> p l o", o=O),
                    fm[:].unsqueeze(2).to_broadcast((BL, L, O)),
                    op=ALU.mult)

                # ---- agreement: u[i,l] = sum_{k,o} w2t ⊙ (v^T x) ----
                if "no_u" in abl:
                    if t == 0:
                        nc.vector.memset(b_acc[:], 0.0)
                    prev_out = None
                    continue
                v16 = v_sb
                u_ps = ps_u.tile([128, NI * L], F32, tag="u_ps")
                u_nat = wpool.tile([128, NI * L], DT, tag="u_nat")
                ar_ins = [dpool.tile([128, (hi - lo) * L], DT,
                                     name=f"ar_in{idx}", tag=f"ar_in{idx}")
                          for idx, (lo, hi) in enumerate(HALVES)]
                ar_outs = [dpool.tile([128, (hi - lo) * L], DT,
                                      name=f"ar_out{idx}", tag=f"ar_out{idx}")
                           for idx, (lo, hi) in enumerate(HALVES)]
                xk = x_nat[:].rearrange("b (k i) -> b k i", k=K)
                for c in range(NI):
                    prod = zpool.tile([F, K * 128], DT, tag="prodT")
                    for h in range(2):
                        t2c = ps_t2.tile([F, 512], F32, tag="t2")
                        nc.tensor.matmul(
                            t2c[:],
                            v16[:],
                            xk[:, 4 * h:4 * h + 4, c * 128:(c + 1) * 128],
                            start=True, stop=True)
                        path = cfg["path"][2 * c + h]
                        w2t_h = w2t[:, (2 * c + h) * 512:(2 * c + h + 1) * 512]
                        prod_h = prod[:, h * 512:(h + 1) * 512]
                        with nc.allow_low_precision("fp16 agr; PE accums f32"):
                            if path == "d":
                                nc.vector.tensor_tensor(
                                    prod_h, t2c[:], w2t_h, op=ALU.mult)
                            else:
                                t2s = zpool.tile([F, 512], DT, tag="t2sT")
                                nc.scalar.activation(t2s[:], t2c[:], AF.Copy)
                                meng = nc.vector if path == "a" else nc.gpsimd
                                meng.tensor_tensor(
                                    prod_h, t2s[:], w2t_h, op=ALU.mult)
                        # reduce over (l,o): accumulating matmuls vs the 0/1
                        # selector; output lands as [i-part, l] (b_acc layout)
                        for kk in range(4):
                            k = 4 * h + kk
                            nc.tensor.matmul(
                                u_ps[:, c * L:(c + 1) * L],
                                prod[:, k * 128:(k + 1) * 128],
                                sel[:],
                                start=(k == 0), stop=(k == K - 1))
                    # flush a finished AR half: convert, bounce, collective
                    if c + 1 in FLUSH:
                        idx, lo, hi = FLUSH[c + 1]
                        cl, ch = lo * L, hi * L
                        nc.scalar.copy(u_nat[:, cl:ch], u_ps[:, cl:ch])
                        nc.sync.dma_start(ar_ins[idx][:], u_nat[:, cl:ch])
                        if "no_ar" in abl:
                            nc.sync.dma_start(ar_outs[idx][:],
                                              ar_ins[idx][:])
                        else:
                            nc.gpsimd.collective_compute(
                                "AllReduce", ALU.add,
                                replica_groups=[list(range(NC))],
                                ins=[ar_ins[idx].opt()],
                                outs=[ar_outs[idx].opt()])

                # keep PE's HAM clock-gate open through the collectives
                if "no_warm" not in abl:
                    wt = ps_sm.tile([BL, 512], F32, tag="sm")
                    for _ in range(cfg["warm"]):
                        nc.tensor.matmul(wt[0:1, 0:NI * L], ones[:], u_nat[:],
                                         start=True, stop=True)
                prev_out = ar_outs

    nc.compile()
    return nc


def _freeze(cfg):
    if not cfg:
        return ()
    return tuple(sorted((k, tuple(v) if isinstance(v, list) else v)
                        for k, v in cfg.items()))


def _get_nc(dt_key, repeat=1, abl=(), cfg=None):
    key = (dt_key, repeat, tuple(sorted(abl)), _freeze(cfg))
    if key not in _CACHE:
        _CACHE[key] = _build(dt_key, repeat, abl, cfg)
    return _CACHE[key]


def make_in_maps(x, w, np_dt):
    w2 = np.ascontiguousarray(w.reshape(I, K * F).astype(np_dt))
    # w2t: [(l,o), (chunk, k, i128)]
    w2t = np.ascontiguousarray(
        w.transpose(2, 3, 1, 0)           # [L, O, K, I]
        .reshape(F, K, NI, 128)
        .transpose(0, 2, 1, 3)            # [F, NI, K, 128]
        .astype(np_dt)).reshape(F, NI * K * 128)
    sel = np.zeros((F, L), dtype=np_dt)
    for l in range(L):
        sel[l * O:(l + 1) * O, l] = 1.0
    in_maps = []
    for c in range(NC):
        xv = x[c * BL:(c + 1) * BL]
        x_nat = np.ascontiguousarray(
            xv.transpose(0, 2, 1).astype(np_dt)).reshape(BL, R)
        x_T = np.ascontiguousarray(
            xv.transpose(2, 1, 0).astype(np_dt)).reshape(R, BL)
        in_maps.append({"x_nat": x_nat, "x_T": x_T, "w2": w2,
                        "w2t": w2t, "sel": sel})
    return in_maps


def kernel(x, w, _dt="f16", _trace=False):
    x = np.asarray(x, dtype=np.float32)
    w = np.asarray(w, dtype=np.float32)
    np_dt = {"f32": np.float32, "f16": np.float16}[_dt]

    nc = _get_nc(_dt)
    in_maps = make_in_maps(x, w, np_dt)

    from concourse.bass_utils import run_bass_kernel_spmd
    res = run_bass_kernel_spmd(
        nc, in_maps, core_ids=list(range(NC)), trace=_trace)
    kernel.last_result = res
    out = np.concatenate([res.results[c]["y"] for c in range(NC)], axis=0)
    return out.astype(np.float32)


kernel.last_result = None


# revision 15
# speedup vs baseline: 1.1811x; 1.0341x over previous
"""Trainium2 Bass kernel for DigitsCapsule dynamic routing — transposed
agreement step + split, compute-overlapped AllReduce.

Per routing iteration:
    s    = x @ (e ⊙ w)               (PE, i-outer so Wc production overlaps)
    v    = squash(s / S[l])           (softmax normalizer folded into squash)
    T2ᵀ  = vᵀ-stationary outer-prod   (PE; out = [(l,o)=112 partitions, (k,i)],
                                       2 strided 512-col matmuls per i-chunk)
    prodᵀ= w2ᵀ ⊙ T2ᵀ                  (per half-chunk: Act/Pool copy + DVE 2x
                                       mult, or direct 1x mult from PSUM)
    u    = Σ_{(l,o)} prodᵀ · Sel      (accumulating PE matmuls against a 0/1
                                       selector; output lands as [i-part, l])
    b   += AllReduce(mean_B u)        (one 8-core collective per iteration;
                                       fp16 [I,L] payload via DRAM bounce)
"""

import numpy as np

B, I, K, L, O = 512, 1152, 8, 16, 7
NC = 8
BL = B // NC          # 64 batch rows per core
R = K * I             # 9216
F = L * O             # 112
NI = I // 128         # 9 partition chunks of I
NH = 2 * NI           # 18 half-chunks (4 k's each)
ITERS = 3

_CACHE = {}

# path per half-chunk (chunk c, half h -> index 2c+h). Pool/DMA cannot
# touch PSUM, so only Act/DVE can evacuate the T2 matmul output:
#   'a' = Act copy + DVE mult       'b' = Act copy + Pool mult
#   'd' = DVE direct mult (PSUM)
DEF_CFG = {
    "path": "abdadbabdadbabdbdd",
    "warm": 14,
    "warmw": 144,
    "wc_pool": 0,      # how many of the 9 Wc chunks run on Pool
}


def _build(dt_key, repeat=1, abl=(), cfg=None):
    """abl: ablation flags for benchmarking — subsets of
    {"no_ar", "no_u", "no_wc", "no_smm", "no_warm"}."""
    import concourse.bacc as bacc
    import concourse.mybir as mybir
    import concourse.tile as tile

    cfg = dict(DEF_CFG, **(cfg or {}))
    assert len(cfg["path"]) == NH

    DT = {"f32": mybir.dt.float32, "f16": mybir.dt.float16}[dt_key]
    F32 = mybir.dt.float32
    AF = mybir.ActivationFunctionType
    ALU = mybir.AluOpType
    AX = mybir.AxisListType

    # finer-grained pipelining of the AR path (the collective itself stays
    # single — NRT AllReduce has ~5.5us fixed cost, so splitting it loses):
    # the return DMA / b-update / e9 / Wc chain advances in thirds, and the
    # outbound bounce starts as soon as a third of the agreement chunks has
    # been reduced
    ABSORB = ((0, 3), (3, 6), (6, NI))
    BOUNCE = {2: (0, 3), 5: (3, 6), 8: (6, NI)}

    nc = bacc.Bacc("TRN2", target_bir_lowering=False, debug=False, num_devices=NC)

    x_nat_d = nc.dram_tensor("x_nat", [BL, R], DT, kind="ExternalInput")
    x_T_d = nc.dram_tensor("x_T", [R, BL], DT, kind="ExternalInput")
    w2_d = nc.dram_tensor("w2", [I, K * F], DT, kind="ExternalInput")
    w2t_d = nc.dram_tensor("w2t", [F, NI * K * 128], DT, kind="ExternalInput")
    sel_d = nc.dram_tensor("sel", [F, L], DT, kind="ExternalInput")
    y_d = nc.dram_tensor("y", [BL, O, L], F32, kind="ExternalOutput")

    with tile.TileContext(nc) as tc:
        with (
            tc.tile_pool(name="const", bufs=1) as cpool,
            tc.tile_pool(name="work", bufs=2) as wpool,
            tc.tile_pool(name="wc", bufs=10) as wcfull,
            tc.tile_pool(name="step6", bufs=4) as zpool,
            tc.tile_pool(name="small", bufs=2) as spool,
            tc.tile_pool(name="ps_s", bufs=1, space="PSUM") as ps_s,
            tc.tile_pool(name="ps_t2", bufs=4, space="PSUM") as ps_t2,
            tc.tile_pool(name="ps_u", bufs=1, space="PSUM") as ps_u,
            tc.tile_pool(name="ps_sm", bufs=1, space="PSUM") as ps_sm,
            tc.tile_pool(name="dram", bufs=2, space="DRAM") as dpool,
        ):
            # ---- load inputs (issue spread across engine DGE queues) ----
            x_nat = cpool.tile([BL, R], DT, tag="x_nat")
            for h in range(2):
                nc.gpsimd.dma_start(x_nat[:, h * R // 2:(h + 1) * R // 2],
                                    x_nat_d[:, h * R // 2:(h + 1) * R // 2])

            # xT tiles: slot t=(k*9+i) holds rows k*1152+i*128 .. +128
            NT = K * NI
            xT = cpool.tile([128, NT * BL], DT, tag="xT")
            xt_src = x_T_d[:].rearrange("(t p) b -> p t b", p=128)
            xt_dst = xT[:].rearrange("p (t b) -> p t b", t=NT)
            for h in range(4):
                lo, hi = h * NT // 4, (h + 1) * NT // 4
                nc.sync.dma_start(xt_dst[:, lo:hi], xt_src[:, lo:hi])

            w2 = cpool.tile([128, NI * K * F], DT, tag="w2")
            w2_src = w2_d[:].rearrange("(i p) f -> p i f", p=128)
            w2_dst = w2[:].rearrange("p (i f) -> p i f", i=NI)
            for h in range(3):
                lo, hi = h * 3, (h + 1) * 3
                nc.scalar.dma_start(w2_dst[:, lo:hi], w2_src[:, lo:hi])

            w2t = cpool.tile([F, NI * K * 128], DT, tag="w2t")
            for h in range(3):
                lo, hi = h * NI * K * 128 // 3, (h + 1) * NI * K * 128 // 3
                nc.scalar.dma_start(w2t[:, lo:hi], w2t_d[:, lo:hi])
            sel = cpool.tile([F, L], DT, tag="sel")
            nc.sync.dma_start(sel[:], sel_d[:])

            ones = cpool.tile([128, 1], DT, tag="ones")
            nc.vector.memset(ones[:], 1.0)
            ones64 = cpool.tile([1, BL], F32, tag="ones64")
            nc.vector.memset(ones64[:], 1.0)

            # warm the PE clock-gate during the input-DMA phase so iteration
            # 0's matmuls run at 2.4 GHz
            WW = cfg["warmw"]
            if "no_warm" not in abl:
                warm0 = cpool.tile([128, WW], DT, tag="warm0")
                nc.vector.memset(warm0[:], 0.0)
                wt0 = ps_sm.tile([BL, 512], F32, tag="sm")
                for _ in range(cfg["warm"]):
                    nc.tensor.matmul(wt0[0:1, 0:WW], ones[:], warm0[:],
                                     start=True, stop=True)


            b_acc = cpool.tile([128, NI * L], F32, tag="b_acc")

            def w2_i(i):
                return w2[:, i * K * F:(i + 1) * K * F]

            for rep in range(repeat):
             prev_out = None
             for t in range(ITERS):
                # ---- absorb previous AR (per half) + e9 + Wc ----
                if t == 0:
                    e9 = None         # e == 1 -> Wc == w2, invS == 1/1152
                    invS_b = None
                    wcs = [w2_i(i) for i in range(NI)]
                else:
                    u_allr = wpool.tile([128, NI * L], DT, tag="u_allr")
                    e9 = wpool.tile([128, NI * F], DT, tag="e9")
                    wcs = []
                    for idx, (lo, hi) in enumerate(ABSORB):
                        cl, ch = lo * L, hi * L
                        nc.sync.dma_start(u_allr[:, cl:ch],
                                          prev_out[:, cl:ch])
                        if t == 1:
                            nc.vector.tensor_copy(b_acc[:, cl:ch],
                                                  u_allr[:, cl:ch])
                        else:
                            nc.vector.tensor_add(b_acc[:, cl:ch],
                                                 b_acc[:, cl:ch],
                                                 u_allr[:, cl:ch])
                        # e replicated over O (contiguous innermost for Wc)
                        nc.scalar.activation(
                            e9[:, lo * F:hi * F]
                            .rearrange("p (jl o) -> p jl o", o=O),
                            b_acc[:, cl:ch]
                            .unsqueeze(2).to_broadcast((128, (hi - lo) * L, O)),
                            AF.Exp, scale=1.0 / B)
                        for i in range(lo, hi):
                            if "no_wc" in abl:
                                wcs.append(w2_i(i))
                                continue
                            wc = wcfull.tile([128, K * F], DT, tag="wc")
                            e_b = (e9[:, i * F:(i + 1) * F]
                                   .unsqueeze(1).to_broadcast((128, K, F)))
                            weng = (nc.gpsimd if i % NI < cfg["wc_pool"]
                                    else nc.vector)
                            weng.tensor_tensor(
                                wc[:].rearrange("p (k f) -> p k f", k=K),
                                w2_i(i).rearrange("p (k f) -> p k f", k=K),
                                e_b, op=ALU.mult)
                            wcs.append(wc[:])

                # ---- s = x @ (e*w), i-outer so it chases Wc production ----
                s_ps = ps_s.tile([BL, F], F32, tag="s_ps")
                if "no_smm" in abl:
                    zz = spool.tile([BL, F], F32, tag="zz")
                    nc.vector.memset(zz[:], 0.0)
                    nc.scalar.activation(s_ps[:], zz[:], AF.Copy)
                else:
                    for i in range(NI):
                        for k in range(K):
                            tslot = k * NI + i
                            nc.tensor.matmul(
                                s_ps[:],
                                xT[:, tslot * BL:(tslot + 1) * BL],
                                wcs[i][:, k * F:(k + 1) * F],
                                start=(i == 0 and k == 0),
                                stop=(i == NI - 1 and k == K - 1))

                # ---- softmax normalizer S (emitted after the s matmuls so
                # its PE ops don't block them in the in-order PE queue) ----
                if t > 0:
                    e_nat = wpool.tile([128, NI * L], DT, tag="e_nat")
                    nc.scalar.activation(e_nat[:], b_acc[:], AF.Exp,
                                         scale=1.0 / B)
                    sm_ps = ps_sm.tile([BL, 512], F32, tag="sm")
                    ssum = sm_ps[0:1, 0:NI * L]
                    nc.tensor.matmul(ssum, ones[:], e_nat[:],
                                     start=True, stop=True)
                    S16 = spool.tile([1, L], F32, tag="S16")
                    nc.vector.tensor_reduce(
                        S16[:], ssum.rearrange("p (i l) -> p l i", i=NI),
                        axis=AX.X, op=ALU.add)
                    invS16 = spool.tile([1, L], F32, tag="invS16")
                    nc.vector.reciprocal(invS16[:], S16[:])
                    bc_ps = sm_ps[0:BL, NI * L:NI * L + L]
                    nc.tensor.matmul(bc_ps, ones64[:], invS16[:],
                                     start=True, stop=True)
                    invS_b = spool.tile([BL, L], F32, tag="invS_b")
                    nc.scalar.copy(invS_b[:], bc_ps)

                # ---- squash (with 1/S[l] folded in) ----
                s_n = wpool.tile([BL, F], F32, tag="s_n")
                if t == 0:
                    nc.vector.tensor_scalar_mul(s_n[:], s_ps[:], 1.0 / I)
                else:
                    nc.vector.tensor_tensor(
                        s_n[:].rearrange("p (l o) -> p l o", o=O),
                        s_ps[:].rearrange("p (l o) -> p l o", o=O),
                        invS_b[:].unsqueeze(2).to_broadcast((BL, L, O)),
                        op=ALU.mult)
                # squash factor: sq/((1+sq)*sqrt(sq)) == sqrt(sq)/(1+sq)
                # (Act order per iter: Exp* -> Square -> [load] Sqrt -> Copy*)
                sq2 = wpool.tile([BL, F], F32, tag="sq2")
                nc.scalar.square(sq2[:], s_n[:])
                sq = spool.tile([BL, L], F32, tag="sq")
                nc.vector.tensor_reduce(
                    sq[:], sq2[:].rearrange("p (l o) -> p l o", o=O),
                    axis=AX.X, op=ALU.add)
                nrm = spool.tile([BL, L], F32, tag="nrm")
                nc.scalar.activation(nrm[:], sq[:], AF.Sqrt)
                d1 = spool.tile([BL, L], F32, tag="d1")
                nc.vector.tensor_scalar_add(d1[:], sq[:], 1.0)
                rin = spool.tile([BL, L], F32, tag="rin")
                nc.vector.reciprocal(rin[:], d1[:])
                fm = spool.tile([BL, L], F32, tag="fm")
                nc.vector.tensor_tensor(fm[:], nrm[:], rin[:], op=ALU.mult)
                if t == ITERS - 1:
                    # fuse the (l,o)->(o,l) output permute into the v multiply
                    v_out = wpool.tile([BL, F], F32, tag="v_out")
                    nc.vector.tensor_tensor(
                        v_out[:].rearrange("p (o l) -> p l o", o=O),
                        s_n[:].rearrange("p (l o) -> p l o", o=O),
                        fm[:].unsqueeze(2).to_broadcast((BL, L, O)),
                        op=ALU.mult)
                    nc.sync.dma_start(y_d[:], v_out[:])
                    continue

                # v in fp16: stationary operand for the T2^T matmuls
                v_sb = wpool.tile([BL, F], DT, tag="v_sb")
                nc.vector.tensor_tensor(
                    v_sb[:].rearrange("p (l o) -> p l o", o=O),
                    s_n[:].rearrange("p (l o) -> p l o", o=O),
                    fm[:].unsqueeze(2).to_broadcast((BL, L, O)),
                    op=ALU.mult)

                # ---- agreement: u[i,l] = sum_{k,o} w2t ⊙ (v^T x) ----
                if "no_u" in abl:
                    if t == 0:
                        nc.vector.memset(b_acc[:], 0.0)
                    prev_out = None
                    continue
                v16 = v_sb
                u_ps = ps_u.tile([128, NI * L], F32, tag="u_ps")
                u_nat = wpool.tile([128, NI * L], DT, tag="u_nat")
                ar_in = dpool.tile([128, NI * L], DT, tag="ar_in")
                ar_out = dpool.tile([128, NI * L], DT, tag="ar_out")
                xk = x_nat[:].rearrange("b (k i) -> b k i", k=K)
                for c in range(NI):
                    prod = zpool.tile([F, K * 128], DT, tag="prodT")
                    for h in range(2):
                        t2c = ps_t2.tile([F, 512], F32, tag="t2")
                        nc.tensor.matmul(
                            t2c[:],
                            v16[:],
                            xk[:, 4 * h:4 * h + 4, c * 128:(c + 1) * 128],
                            start=True, stop=True)
                        path = cfg["path"][2 * c + h]
                        w2t_h = w2t[:, (2 * c + h) * 512:(2 * c + h + 1) * 512]
                        prod_h = prod[:, h * 512:(h + 1) * 512]
                        with nc.allow_low_precision("fp16 agr; PE accums f32"):
                            if path == "d":
                                nc.vector.tensor_tensor(
                                    prod_h, t2c[:], w2t_h, op=ALU.mult)
                            else:
                                t2s = zpool.tile([F, 512], DT, tag="t2sT")
                                nc.scalar.activation(t2s[:], t2c[:], AF.Copy)
                                meng = nc.vector if path == "a" else nc.gpsimd
                                meng.tensor_tensor(
                                    prod_h, t2s[:], w2t_h, op=ALU.mult)
                        # reduce over (l,o): accumulating matmuls vs the 0/1
                        # selector; output lands as [i-part, l] (b_acc layout)
                        for kk in range(4):
                            k = 4 * h + kk
                            nc.tensor.matmul(
                                u_ps[:, c * L:(c + 1) * L],
                                prod[:, k * 128:(k + 1) * 128],
                                sel[:],
                                start=(k == 0), stop=(k == K - 1))
                    # bounce finished thirds out early; collective fires once
                    if c in BOUNCE:
                        lo, hi = BOUNCE[c]
                        cl, ch = lo * L, hi * L
                        nc.scalar.copy(u_nat[:, cl:ch], u_ps[:, cl:ch])
                        nc.sync.dma_start(ar_in[:, cl:ch], u_nat[:, cl:ch])
                    if c == NI - 1:
                        if "no_ar" in abl:
                            nc.sync.dma_start(ar_out[:], ar_in[:])
                        else:
                            nc.gpsimd.collective_compute(
                                "AllReduce", ALU.add,
                                replica_groups=[list(range(NC))],
                                ins=[ar_in.opt()], outs=[ar_out.opt()])

                # keep PE's HAM clock-gate open through the collectives
                if "no_warm" not in abl:
                    wt = ps_sm.tile([BL, 512], F32, tag="sm")
                    for _ in range(cfg["warm"]):
                        nc.tensor.matmul(wt[0:1, 0:NI * L], ones[:], u_nat[:],
                                         start=True, stop=True)
                prev_out = ar_out

    nc.compile()
    return nc


def _freeze(cfg):
    if not cfg:
        return ()
    return tuple(sorted((k, tuple(v) if isinstance(v, list) else v)
                        for k, v in cfg.items()))


def _get_nc(dt_key, repeat=1, abl=(), cfg=None):
    key = (dt_key, repeat, tuple(sorted(abl)), _freeze(cfg))
    if key not in _CACHE:
        _CACHE[key] = _build(dt_key, repeat, abl, cfg)
    return _CACHE[key]


def make_in_maps(x, w, np_dt):
    w2 = np.ascontiguousarray(w.reshape(I, K * F).astype(np_dt))
    # w2t: [(l,o), (chunk, k, i128)]
    w2t = np.ascontiguousarray(
        w.transpose(2, 3, 1, 0)           # [L, O, K, I]
        .reshape(F, K, NI, 128)
        .transpose(0, 2, 1, 3)            # [F, NI, K, 128]
        .astype(np_dt)).reshape(F, NI * K * 128)
    sel = np.zeros((F, L), dtype=np_dt)
    for l in range(L):
        sel[l * O:(l + 1) * O, l] = 1.0
    in_maps = []
    for c in range(NC):
        xv = x[c * BL:(c + 1) * BL]
        x_nat = np.ascontiguousarray(
            xv.transpose(0, 2, 1).astype(np_dt)).reshape(BL, R)
        x_T = np.ascontiguousarray(
            xv.transpose(2, 1, 0).astype(np_dt)).reshape(R, BL)
        in_maps.append({"x_nat": x_nat, "x_T": x_T, "w2": w2,
                        "w2t": w2t, "sel": sel})
    return in_maps


def kernel(x, w, _dt="f16", _trace=False):
    x = np.asarray(x, dtype=np.float32)
    w = np.asarray(w, dtype=np.float32)
    np_dt = {"f32": np.float32, "f16": np.float16}[_dt]

    nc = _get_nc(_dt)
    in_maps = make_in_maps(x, w, np_dt)

    from concourse.bass_utils import run_bass_kernel_spmd
    res = run_bass_kernel_spmd(
        nc, in_maps, core_ids=list(range(NC)), trace=_trace)
    kernel.last_result = res
    out = np.concatenate([res.results[c]["y"] for c in range(NC)], axis=0)
    return out.astype(np.float32)


kernel.last_result = None
